# revision 34
# baseline (speedup 1.0000x reference)
"""Trainium2 Bass kernel for nn_AnchorKNN_OnlyL (retrieval_knn).

Per query b (32768 total): among its own 2048 2-D anchors, find the 8 nearest
(L2), run each through a 2->128->128 erf-GELU MLP, and combine with
softmax(d2_top / tau) weights.

Sharding: pure data parallel over queries -- 4096 queries per NeuronCore,
params replicated, no collectives.

Per-core algorithm (tiles of 128 queries x 2048 anchors):
  - ACT: squares (x-gx)^2, (y-gy)^2 fused sub+square
  - s = -d2 via scalar_tensor_tensor
  - DVE InstMax/InstMaxIndex: top-8 values + indices per partition row
  - per-(query,k) anchor coords fetched with gpsimd dma_gather (256B chunks
    from DRAM; per-row index list built with two re-layout DMAs), then picked
    out of each 64-float chunk with an iota-compare + masked pair-reduce
  - MLP on TensorE (layer1 via 8 block stationaries with contraction 16,
    layer2 bf16 128x128), GELU + biases on ACT
  - softmax weights via tanh-based exp (stays in the 'gelu' ACT table set),
    broadcast across partitions with one-hot-row stationaries, weighted sum
    via multiply + strided reduce, final transpose + 1/denom scale on the
    way out
"""

import sys

sys.path.insert(0, "/opt/trn_rl_repo")

import numpy as np

B, M, EMB, K = 32768, 2048, 128, 8
TAU = 0.3
NCORES = 8
BP = B // NCORES          # queries per core
P = 128                   # partitions / queries per tile
NT_FULL = BP // P         # 32 tiles per core
CH = 64                   # wrapped index-list width (1024 idxs / 16 rows)
CHF = 64                  # floats per gathered chunk (256 B)
NCHUNK = (M * 2) // CHF   # 64 chunks per query row


def host_prep(W1, b1, W2, b2):
    """Derived replicated parameters computed host-side."""
    w1T = np.ascontiguousarray(W1.T)                      # [2, EMB]
    w1big = np.zeros((2 * K, K * EMB), dtype=np.float32)  # 8 block stationaries
    for k in range(K):
        w1big[2 * k : 2 * k + 2, k * EMB : (k + 1) * EMB] = w1T
    onesel = np.zeros((K, K * P), dtype=np.float32)       # one-hot row selectors
    for k in range(K):
        onesel[k, k * P : (k + 1) * P] = 1.0
    w2t = np.ascontiguousarray(W2.T)                      # [EMB, EMB]
    repident = np.zeros((16, P), dtype=np.float32)        # S[r, p] = (p%16 == r)
    for p in range(P):
        repident[p % 16, p] = 1.0
    return {
        "repident": repident,
        "w1big": w1big,
        "onesel": onesel,
        "w2t": w2t,
        "b1c": np.ascontiguousarray(b1.reshape(EMB, 1)),
        "b2c": np.ascontiguousarray(b2.reshape(EMB, 1)),
    }


def shard_queries(Gl_cur, n_tiles):
    """[bp, 2] -> negated, tile-major layout [P, n_tiles, 2]."""
    g = Gl_cur.reshape(n_tiles, P, 2).transpose(1, 0, 2)
    return np.ascontiguousarray(-g)


def build_graph(n_tiles=NT_FULL):
    import concourse.bass as bass
    import concourse.mybir as mybir
    from concourse.bass import ds, ts
    from concourse.masks import make_identity
    from concourse.tile import TileContext

    f32 = mybir.dt.float32
    bf16 = mybir.dt.bfloat16
    i32 = mybir.dt.int32
    i16 = mybir.dt.int16
    u16 = mybir.dt.uint16
    Alu = mybir.AluOpType
    Act = mybir.ActivationFunctionType

    from concourse import bacc

    bp = n_tiles * P
    nc = bacc.Bacc(None, dynamic_dma_scratch_size=16384, num_swdge_queues=2)

    ngl_ext = nc.declare_dram_parameter("ngl", [P, n_tiles, 2], f32, isOutput=False)
    anc_ext = nc.declare_dram_parameter("anc", [bp, M, 2], f32, isOutput=False)
    w1big_ext = nc.declare_dram_parameter("w1big", [2 * K, K * EMB], f32, isOutput=False)
    onesel_ext = nc.declare_dram_parameter("onesel", [K, K * P], f32, isOutput=False)
    w2t_ext = nc.declare_dram_parameter("w2t", [EMB, EMB], f32, isOutput=False)
    b1_ext = nc.declare_dram_parameter("b1c", [EMB, 1], f32, isOutput=False)
    repident_ext = nc.declare_dram_parameter("repident", [16, P], f32, isOutput=False)
    b2_ext = nc.declare_dram_parameter("b2c", [EMB, 1], f32, isOutput=False)
    out_ext = nc.declare_dram_parameter("out", [bp, EMB], f32, isOutput=True)

    with TileContext(nc) as tc:
        with (
            tc.tile_pool(name="consts", bufs=1) as consts,
            tc.tile_pool(name="anc", bufs=3) as anc_pool,
            tc.tile_pool(name="sq", bufs=2) as sq_pool,
            tc.tile_pool(name="sel", bufs=5) as sel_pool,
            tc.tile_pool(name="mid", bufs=4) as mid_pool,
            tc.tile_pool(name="mlp", bufs=4) as mlp_pool,
            tc.tile_pool(name="psum_big", bufs=1, space="PSUM") as ppool,
            tc.tile_pool(name="psum_small", bufs=1, space="PSUM") as pspool,
            tc.tile_pool(name="dram", bufs=4, space="DRAM") as dram_pool,
        ):
            # ---------------- constants ----------------
            ident = consts.tile([P, P], f32)
            make_identity(nc, ident)

            w1big = consts.tile([2 * K, K * EMB], f32)
            nc.sync.dma_start(out=w1big, in_=w1big_ext[:, :])
            onesel = consts.tile([K, K * P], f32)
            nc.sync.dma_start(out=onesel, in_=onesel_ext[:, :])

            w2sb = consts.tile([EMB, EMB], f32)
            nc.sync.dma_start(out=w2sb, in_=w2t_ext[:, :])
            w2T = consts.tile([EMB, EMB], bf16)
            nc.scalar.copy(w2T, w2sb)

            b1c = consts.tile([EMB, 1], f32)
            nc.sync.dma_start(out=b1c, in_=b1_ext[:, :])
            b2c = consts.tile([EMB, 1], f32)
            nc.sync.dma_start(out=b2c, in_=b2_ext[:, :])

            repident = consts.tile([16, P], f32)
            nc.sync.dma_start(out=repident, in_=repident_ext[:, :])

            ngl = consts.tile([P, n_tiles, 2], f32)
            nc.sync.dma_start(out=ngl, in_=ngl_ext[:, :, :])

            # jhalf[p, j] = j // 2 for j in [0, 128)
            jhalf = consts.tile([P, CHF], u16)
            nc.gpsimd.iota(jhalf, pattern=[[1, CHF // 2], [0, 2]], channel_multiplier=0)

            # qiota[p, k] = 64 * p  (chunk-row base for query p within its tile)
            qiota = consts.tile([P, K], u16)
            nc.gpsimd.iota(qiota, pattern=[[0, K]], channel_multiplier=NCHUNK)

            def stage_a(t):
                """load, distances, top-8, index fold, gather issue, softmax."""
                ancd = anc_pool.tile([P, M * 2], f32, tag="ancd")
                nc.sync.dma_start(
                    out=ancd, in_=anc_ext[ts(t, P)].rearrange("p m c -> p (m c)")
                )
                anc_x = ancd.rearrange("p (m c) -> p c m", c=2)[:, 0]
                anc_y = ancd.rearrange("p (m c) -> p c m", c=2)[:, 1]

                tx2 = sq_pool.tile([P, M], f32, tag="tx2")
                ty2 = sq_pool.tile([P, M], f32, tag="ty2")
                nc.scalar.activation(tx2, anc_x, Act.Square, bias=ngl[:, t, 0:1])
                nc.scalar.activation(ty2, anc_y, Act.Square, bias=ngl[:, t, 1:2])

                s = sq_pool.tile([P, M], f32, tag="s")
                nc.vector.scalar_tensor_tensor(
                    out=s, in0=tx2, scalar=-1.0, in1=ty2,
                    op0=Alu.mult, op1=Alu.subtract,
                )

                vals8 = sel_pool.tile([P, K], f32, tag="vals8")
                nc.vector.max(out=vals8, in_=s)
                idx8 = sel_pool.tile([P, K], u16, tag="idx8")
                nc.vector.max_index(out=idx8, in_max=vals8, in_values=s)

                # chunk row index within this tile's [8192, 64] view of ancL
                chunk = sel_pool.tile([P, K], u16, tag="chunk")
                nc.vector.tensor_scalar(
                    out=chunk, in0=idx8, scalar1=5, scalar2=None,
                    op0=Alu.logical_shift_right,
                )
                chunkq = sel_pool.tile([P, K], u16, tag="chunkq")
                nc.vector.tensor_tensor(out=chunkq, in0=chunk, in1=qiota, op=Alu.add)

                # wrapped index list for dma_gather, built on-chip:
                # wrapped[r, 8k+j] = chunkq[16j+r, k] replicated to the 8 cores.
                chunkqf = sel_pool.tile([P, K], f32, tag="chunkqf")
                nc.scalar.copy(chunkqf, chunkq)
                t1_ps = pspool.tile([K, P], f32, tag="fold_ps")
                nc.tensor.transpose(t1_ps, chunkqf, ident)
                t1_sb = sel_pool.tile([K, P], f32, tag="t1_sb")
                nc.scalar.copy(t1_sb, t1_ps)
                m16_ps = pspool.tile([16, CH], f32, tag="fold_ps")
                for j in range(8):
                    nc.tensor.transpose(
                        m16_ps[:, ts(j, K)],
                        t1_sb[:, ds(16 * j, 16)],
                        ident[ds(0, K), ds(0, K)],
                    )
                m16_sb = sel_pool.tile([16, CH], f32, tag="m16_sb")
                nc.scalar.copy(m16_sb, m16_ps)
                wrapped_ps = pspool.tile([P, CH], f32, tag="fold_ps")
                nc.tensor.matmul(
                    wrapped_ps,
                    repident,
                    m16_sb.rearrange("r (j k) -> r k j", k=K, j=K),
                )
                wrapped = sel_pool.tile([P, CH], i16, tag="wrapped")
                nc.scalar.copy(wrapped, wrapped_ps)

                # gather 256B chunks: chunks[p, k, :] = anc row-chunk of (q=p, k)
                chunks = mid_pool.tile([P, K * CHF], f32, tag="chunks")
                nc.gpsimd.dma_gather(
                    out_ap=chunks.rearrange("p (k e) -> p k e", e=CHF),
                    in_ap=anc_ext[ts(t, P)].rearrange(
                        "p (g r) c -> (p g) (r c)", r=CHF // 2
                    ),
                    idxs_ap=wrapped,
                    num_idxs=P * K,
                    num_idxs_reg=P * K,
                    elem_size=CHF,
                    queue_num=t % 2,
                )

                # chunk-local selection mask (doesn't need the gather result)
                loc16 = sel_pool.tile([P, K], u16, tag="loc16")
                nc.vector.tensor_scalar(
                    out=loc16, in0=idx8, scalar1=31, scalar2=None,
                    op0=Alu.bitwise_and,
                )
                m_ = mid_pool.tile([P, K * CHF], f32, tag="m_")
                nc.vector.tensor_tensor(
                    out=m_.rearrange("p (k j) -> p k j", k=K),
                    in0=jhalf[:, None, :].broadcast_to([P, K, CHF]),
                    in1=loc16[:, :, None].broadcast_to([P, K, CHF]),
                    op=Alu.is_equal,
                )

                # softmax weights via tanh-exp; broadcast via DRAM bounce
                sub = sel_pool.tile([P, K], f32, tag="sub")
                nc.vector.tensor_tensor(
                    out=sub, in0=vals8,
                    in1=vals8[:, 7:8].broadcast_to([P, K]),
                    op=Alu.subtract,
                )
                th = sel_pool.tile([P, K], f32, tag="th")
                nc.scalar.activation(th, sub, Act.Tanh, scale=-1.0 / (2.0 * TAU))
                den = sel_pool.tile([P, K], f32, tag="den")
                nc.vector.tensor_scalar(
                    out=den, in0=th, scalar1=-1.0, scalar2=1.0,
                    op0=Alu.mult, op1=Alu.add,
                )
                rden8 = sel_pool.tile([P, K], f32, tag="rden8")
                nc.vector.reciprocal(rden8, den)
                exp8 = sel_pool.tile([P, K], f32, tag="exp8")
                denom = sel_pool.tile([P, 1], f32, tag="denom")
                nc.vector.scalar_tensor_tensor(
                    out=exp8, in0=th, scalar=1.0, in1=rden8,
                    op0=Alu.add, op1=Alu.mult, accum_out=denom,
                )
                rden = sel_pool.tile([P, 1], f32, tag="rden")
                nc.vector.reciprocal(rden, denom)

                expT_ps = pspool.tile([K, P], f32, tag="tp_ps")
                nc.tensor.transpose(expT_ps, exp8, ident)
                expT = sel_pool.tile([K, P], bf16, tag="expT")
                nc.scalar.copy(expT, expT_ps)
                wscr = dram_pool.tile([K, P], bf16, tag="wscr")
                nc.sync.dma_start(out=wscr, in_=expT)
                wrep = mlp_pool.tile([EMB, K * P], bf16, tag="wrep")
                nc.sync.dma_start(
                    out=wrep,
                    in_=wscr[None, :, :].broadcast_to([EMB, K, P]),
                )
                return dict(chunks=chunks, m_=m_, wrep=wrep, rden=rden)

            def stage_b(t, st):
                """extract coords, MLP, weighted sum, store."""
                mx = mid_pool.tile([P, K * CHF], f32, tag="mx")
                nc.vector.tensor_tensor(
                    out=mx, in0=st["m_"], in1=st["chunks"], op=Alu.mult
                )
                topA2 = sel_pool.tile([P, 2 * K], f32, tag="topA2")
                nc.vector.tensor_reduce(
                    out=topA2.rearrange("p (k c) -> p k c", c=2),
                    in_=mx.rearrange("p (k j32 c) -> p k c j32", k=K, c=2),
                    axis=mybir.AxisListType.X,
                    op=Alu.add,
                )

                topAT_ps = pspool.tile([2 * K, P], f32, tag="topat_ps")
                nc.tensor.transpose(topAT_ps, topA2, ident)
                topAT = sel_pool.tile([2 * K, P], f32, tag="topAT")
                nc.scalar.copy(topAT, topAT_ps)

                psum1 = ppool.tile([EMB, K * P], f32, tag="pbig1")
                for k in range(K):
                    nc.tensor.matmul(
                        psum1[:, ts(k, P)], w1big[:, ts(k, EMB)], topAT
                    )
                h1 = mlp_pool.tile([EMB, K * P], bf16, tag="h1")
                nc.scalar.activation(h1, psum1, Act.Gelu, bias=b1c)

                psum2 = ppool.tile([EMB, K * P], f32, tag="pbig2")
                nc.tensor.matmul(psum2[:, :512], w2T, h1[:, :512])
                nc.tensor.matmul(psum2[:, 512:], w2T, h1[:, 512:])
                topE = mlp_pool.tile([EMB, K * P], bf16, tag="topE")
                nc.scalar.activation(topE, psum2, Act.Gelu, bias=b2c)

                wtmp = mlp_pool.tile([EMB, K * P], bf16, tag="wtmp")
                nc.vector.tensor_tensor(
                    out=wtmp, in0=topE, in1=st["wrep"], op=Alu.mult
                )
                f1 = mlp_pool.tile([EMB, 4 * P], bf16, tag="f1")
                nc.vector.tensor_tensor(
                    out=f1, in0=wtmp[:, : 4 * P], in1=wtmp[:, 4 * P :], op=Alu.add
                )
                f2 = sel_pool.tile([EMB, 2 * P], bf16, tag="f2")
                nc.vector.tensor_tensor(
                    out=f2, in0=f1[:, : 2 * P], in1=f1[:, 2 * P :], op=Alu.add
                )
                outT = sel_pool.tile([EMB, P], f32, tag="outT")
                nc.vector.tensor_tensor(
                    out=outT, in0=f2[:, :P], in1=f2[:, P:], op=Alu.add
                )

                outQ_ps = pspool.tile([P, EMB], f32, tag="outq_ps")
                nc.tensor.transpose(outQ_ps, outT, ident)
                out_sb = sel_pool.tile([P, EMB], f32, tag="out_sb")
                nc.scalar.mul(out_sb, outQ_ps, mul=st["rden"])

                nc.sync.dma_start(out=out_ext[ts(t, P), :], in_=out_sb)

            DEPTH = 4
            state = {}
            for t in range(n_tiles + DEPTH):
                if t < n_tiles:
                    state[t] = stage_a(t)
                if t >= DEPTH:
                    stage_b(t - DEPTH, state.pop(t - DEPTH))

    nc.compile()
    return nc


def make_in_map(gl_shard, anc_shard, prep, n_tiles):
    m = {
        "ngl": shard_queries(gl_shard, n_tiles),
        "anc": anc_shard,
    }
    m.update(prep)
    return m


_GRAPH_CACHE = {}
_TRACE = False       # set by test harnesses to capture a profile
LAST_RESULT = None   # BassKernelResults of the most recent kernel() call


def kernel(Gl_cur, ancL, W1, b1, W2, b2):
    global LAST_RESULT
    from concourse.bass_utils import run_bass_kernel_spmd

    Gl_cur = np.ascontiguousarray(Gl_cur, dtype=np.float32)
    ancL = np.ascontiguousarray(ancL, dtype=np.float32)
    prep = host_prep(
        np.asarray(W1, dtype=np.float32),
        np.asarray(b1, dtype=np.float32),
        np.asarray(W2, dtype=np.float32),
        np.asarray(b2, dtype=np.float32),
    )

    if "nc" not in _GRAPH_CACHE:
        _GRAPH_CACHE["nc"] = build_graph(NT_FULL)
    nc = _GRAPH_CACHE["nc"]

    in_maps = []
    for i in range(NCORES):
        sl = slice(i * BP, (i + 1) * BP)
        in_maps.append(make_in_map(Gl_cur[sl], ancL[sl], prep, NT_FULL))
    res = run_bass_kernel_spmd(nc, in_maps, list(range(NCORES)), trace=_TRACE)
    LAST_RESULT = res
    return np.concatenate([res.results[i]["out"] for i in range(NCORES)], axis=0)


# revision 36
# speedup vs baseline: 1.0156x; 1.0156x over previous
"""Trainium2 Bass kernel for nn_AnchorKNN_OnlyL (retrieval_knn).

Per query b (32768 total): among its own 2048 2-D anchors, find the 8 nearest
(L2), run each through a 2->128->128 erf-GELU MLP, and combine with
softmax(d2_top / tau) weights.

Sharding: pure data parallel over queries -- 4096 queries per NeuronCore,
params replicated, no collectives.

Per-core algorithm (tiles of 128 queries x 2048 anchors):
  - ACT: squares (x-gx)^2, (y-gy)^2 fused sub+square
  - s = -d2 via scalar_tensor_tensor
  - DVE InstMax/InstMaxIndex: top-8 values + indices per partition row
  - per-(query,k) anchor coords fetched with gpsimd dma_gather (256B chunks
    from DRAM; per-row index list built with two re-layout DMAs), then picked
    out of each 64-float chunk with an iota-compare + masked pair-reduce
  - MLP on TensorE (layer1 via 8 block stationaries with contraction 16,
    layer2 bf16 128x128), GELU + biases on ACT
  - softmax weights via tanh-based exp (stays in the 'gelu' ACT table set),
    broadcast across partitions with one-hot-row stationaries, weighted sum
    via multiply + strided reduce, final transpose + 1/denom scale on the
    way out
"""

import sys

sys.path.insert(0, "/opt/trn_rl_repo")

import numpy as np

B, M, EMB, K = 32768, 2048, 128, 8
TAU = 0.3
NCORES = 8
BP = B // NCORES          # queries per core
P = 128                   # partitions / queries per tile
NT_FULL = BP // P         # 32 tiles per core
CH = 64                   # wrapped index-list width (1024 idxs / 16 rows)
CHF = 64                  # floats per gathered chunk (256 B)
NCHUNK = (M * 2) // CHF   # 64 chunks per query row


def host_prep(W1, b1, W2, b2):
    """Derived replicated parameters computed host-side."""
    w1T = np.ascontiguousarray(W1.T)                      # [2, EMB]
    w1big = np.zeros((2 * K, K * EMB), dtype=np.float32)  # 8 block stationaries
    for k in range(K):
        w1big[2 * k : 2 * k + 2, k * EMB : (k + 1) * EMB] = w1T
    onesel = np.zeros((K, K * P), dtype=np.float32)       # one-hot row selectors
    for k in range(K):
        onesel[k, k * P : (k + 1) * P] = 1.0
    w2t = np.ascontiguousarray(W2.T)                      # [EMB, EMB]
    repident = np.zeros((16, P), dtype=np.float32)        # S[r, p] = (p%16 == r)
    for p in range(P):
        repident[p % 16, p] = 1.0
    return {
        "repident": repident,
        "w1big": w1big,
        "onesel": onesel,
        "w2t": w2t,
        "b1c": np.ascontiguousarray(b1.reshape(EMB, 1)),
        "b2c": np.ascontiguousarray(b2.reshape(EMB, 1)),
    }


def shard_queries(Gl_cur, n_tiles):
    """[bp, 2] -> negated, tile-major layout [P, n_tiles, 2]."""
    g = Gl_cur.reshape(n_tiles, P, 2).transpose(1, 0, 2)
    return np.ascontiguousarray(-g)


def build_graph(n_tiles=NT_FULL):
    import concourse.bass as bass
    import concourse.mybir as mybir
    from concourse.bass import ds, ts
    from concourse.masks import make_identity
    from concourse.tile import TileContext

    f32 = mybir.dt.float32
    bf16 = mybir.dt.bfloat16
    i32 = mybir.dt.int32
    i16 = mybir.dt.int16
    u16 = mybir.dt.uint16
    Alu = mybir.AluOpType
    Act = mybir.ActivationFunctionType

    from concourse import bacc

    bp = n_tiles * P
    nc = bacc.Bacc(None, dynamic_dma_scratch_size=16384, num_swdge_queues=2)

    ngl_ext = nc.declare_dram_parameter("ngl", [P, n_tiles, 2], f32, isOutput=False)
    anc_ext = nc.declare_dram_parameter("anc", [bp, M, 2], f32, isOutput=False)
    w1big_ext = nc.declare_dram_parameter("w1big", [2 * K, K * EMB], f32, isOutput=False)
    onesel_ext = nc.declare_dram_parameter("onesel", [K, K * P], f32, isOutput=False)
    w2t_ext = nc.declare_dram_parameter("w2t", [EMB, EMB], f32, isOutput=False)
    b1_ext = nc.declare_dram_parameter("b1c", [EMB, 1], f32, isOutput=False)
    repident_ext = nc.declare_dram_parameter("repident", [16, P], f32, isOutput=False)
    b2_ext = nc.declare_dram_parameter("b2c", [EMB, 1], f32, isOutput=False)
    out_ext = nc.declare_dram_parameter("out", [bp, EMB], f32, isOutput=True)

    with TileContext(nc) as tc:
        with (
            tc.tile_pool(name="consts", bufs=1) as consts,
            tc.tile_pool(name="anc", bufs=3) as anc_pool,
            tc.tile_pool(name="sq", bufs=2) as sq_pool,
            tc.tile_pool(name="sel", bufs=4) as sel_pool,
            tc.tile_pool(name="mid", bufs=3) as mid_pool,
            tc.tile_pool(name="mlp", bufs=3) as mlp_pool,
            tc.tile_pool(name="psum_big", bufs=1, space="PSUM") as ppool,
            tc.tile_pool(name="psum_small", bufs=1, space="PSUM") as pspool,
            tc.tile_pool(name="dram", bufs=4, space="DRAM") as dram_pool,
        ):
            # ---------------- constants ----------------
            ident = consts.tile([P, P], f32)
            make_identity(nc, ident)

            w1big = consts.tile([2 * K, K * EMB], f32)
            nc.sync.dma_start(out=w1big, in_=w1big_ext[:, :])
            onesel = consts.tile([K, K * P], f32)
            nc.sync.dma_start(out=onesel, in_=onesel_ext[:, :])

            w2sb = consts.tile([EMB, EMB], f32)
            nc.sync.dma_start(out=w2sb, in_=w2t_ext[:, :])
            w2T = consts.tile([EMB, EMB], bf16)
            nc.scalar.copy(w2T, w2sb)

            b1c = consts.tile([EMB, 1], f32)
            nc.sync.dma_start(out=b1c, in_=b1_ext[:, :])
            b2c = consts.tile([EMB, 1], f32)
            nc.sync.dma_start(out=b2c, in_=b2_ext[:, :])

            repident = consts.tile([16, P], f32)
            nc.sync.dma_start(out=repident, in_=repident_ext[:, :])

            ngl = consts.tile([P, n_tiles, 2], f32)
            nc.sync.dma_start(out=ngl, in_=ngl_ext[:, :, :])

            # jhalf[p, j] = j // 2 for j in [0, 128)
            jhalf = consts.tile([P, CHF], u16)
            nc.gpsimd.iota(jhalf, pattern=[[1, CHF // 2], [0, 2]], channel_multiplier=0)

            # qiota[p, k] = 64 * p  (chunk-row base for query p within its tile)
            qiota = consts.tile([P, K], u16)
            nc.gpsimd.iota(qiota, pattern=[[0, K]], channel_multiplier=NCHUNK)

            def stage_a(t):
                """load, distances, top-8, index fold, gather issue, softmax."""
                ancd = anc_pool.tile([P, M * 2], f32, tag="ancd")
                nc.sync.dma_start(
                    out=ancd, in_=anc_ext[ts(t, P)].rearrange("p m c -> p (m c)")
                )
                anc_x = ancd.rearrange("p (m c) -> p c m", c=2)[:, 0]
                anc_y = ancd.rearrange("p (m c) -> p c m", c=2)[:, 1]

                tx2 = sq_pool.tile([P, M], f32, tag="tx2")
                ty2 = sq_pool.tile([P, M], f32, tag="ty2")
                nc.scalar.activation(tx2, anc_x, Act.Square, bias=ngl[:, t, 0:1])
                nc.scalar.activation(ty2, anc_y, Act.Square, bias=ngl[:, t, 1:2])

                s = sq_pool.tile([P, M], f32, tag="s")
                nc.vector.scalar_tensor_tensor(
                    out=s, in0=tx2, scalar=-1.0, in1=ty2,
                    op0=Alu.mult, op1=Alu.subtract,
                )

                vals8 = sel_pool.tile([P, K], f32, tag="vals8")
                nc.vector.max(out=vals8, in_=s)
                idx8 = sel_pool.tile([P, K], u16, tag="idx8")
                nc.vector.max_index(out=idx8, in_max=vals8, in_values=s)

                # chunk row index within this tile's [8192, 64] view of ancL
                chunk = sel_pool.tile([P, K], u16, tag="chunk")
                nc.vector.tensor_scalar(
                    out=chunk, in0=idx8, scalar1=5, scalar2=None,
                    op0=Alu.logical_shift_right,
                )
                chunkq = sel_pool.tile([P, K], u16, tag="chunkq")
                nc.vector.tensor_tensor(out=chunkq, in0=chunk, in1=qiota, op=Alu.add)

                # wrapped index list for dma_gather, built on-chip:
                # wrapped[r, 8k+j] = chunkq[16j+r, k] replicated to the 8 cores.
                chunkqf = sel_pool.tile([P, K], f32, tag="chunkqf")
                nc.scalar.copy(chunkqf, chunkq)
                t1_ps = pspool.tile([K, P], f32, tag="fold_ps")
                nc.tensor.transpose(t1_ps, chunkqf, ident)
                t1_sb = sel_pool.tile([K, P], f32, tag="t1_sb")
                nc.scalar.copy(t1_sb, t1_ps)
                m16_ps = pspool.tile([16, CH], f32, tag="fold_ps")
                for j in range(8):
                    nc.tensor.transpose(
                        m16_ps[:, ts(j, K)],
                        t1_sb[:, ds(16 * j, 16)],
                        ident[ds(0, K), ds(0, K)],
                    )
                m16_sb = sel_pool.tile([16, CH], f32, tag="m16_sb")
                nc.scalar.copy(m16_sb, m16_ps)
                wrapped_ps = pspool.tile([P, CH], f32, tag="fold_ps")
                nc.tensor.matmul(
                    wrapped_ps,
                    repident,
                    m16_sb.rearrange("r (j k) -> r k j", k=K, j=K),
                )
                wrapped = sel_pool.tile([P, CH], i16, tag="wrapped")
                nc.scalar.copy(wrapped, wrapped_ps)

                # gather 256B chunks: chunks[p, k, :] = anc row-chunk of (q=p, k)
                chunks = mid_pool.tile([P, K * CHF], f32, tag="chunks")
                nc.gpsimd.dma_gather(
                    out_ap=chunks.rearrange("p (k e) -> p k e", e=CHF),
                    in_ap=anc_ext[ts(t, P)].rearrange(
                        "p (g r) c -> (p g) (r c)", r=CHF // 2
                    ),
                    idxs_ap=wrapped,
                    num_idxs=P * K,
                    num_idxs_reg=P * K,
                    elem_size=CHF,
                    queue_num=t % 2,
                )

                # chunk-local selection mask (doesn't need the gather result)
                loc16 = sel_pool.tile([P, K], u16, tag="loc16")
                nc.vector.tensor_scalar(
                    out=loc16, in0=idx8, scalar1=31, scalar2=None,
                    op0=Alu.bitwise_and,
                )
                m_ = mid_pool.tile([P, K * CHF], f32, tag="m_")
                nc.vector.tensor_tensor(
                    out=m_.rearrange("p (k j) -> p k j", k=K),
                    in0=jhalf[:, None, :].broadcast_to([P, K, CHF]),
                    in1=loc16[:, :, None].broadcast_to([P, K, CHF]),
                    op=Alu.is_equal,
                )

                # softmax weights via tanh-exp; broadcast via DRAM bounce
                sub = sel_pool.tile([P, K], f32, tag="sub")
                nc.vector.tensor_tensor(
                    out=sub, in0=vals8,
                    in1=vals8[:, 7:8].broadcast_to([P, K]),
                    op=Alu.subtract,
                )
                th = sel_pool.tile([P, K], f32, tag="th")
                nc.scalar.activation(th, sub, Act.Tanh, scale=-1.0 / (2.0 * TAU))
                den = sel_pool.tile([P, K], f32, tag="den")
                nc.vector.tensor_scalar(
                    out=den, in0=th, scalar1=-1.0, scalar2=1.0,
                    op0=Alu.mult, op1=Alu.add,
                )
                rden8 = sel_pool.tile([P, K], f32, tag="rden8")
                nc.vector.reciprocal(rden8, den)
                exp8 = sel_pool.tile([P, K], f32, tag="exp8")
                denom = sel_pool.tile([P, 1], f32, tag="denom")
                nc.vector.scalar_tensor_tensor(
                    out=exp8, in0=th, scalar=1.0, in1=rden8,
                    op0=Alu.add, op1=Alu.mult, accum_out=denom,
                )
                rden = sel_pool.tile([P, 1], f32, tag="rden")
                nc.vector.reciprocal(rden, denom)

                expT_ps = pspool.tile([K, P], f32, tag="tp_ps")
                nc.tensor.transpose(expT_ps, exp8, ident)
                expT = sel_pool.tile([K, P], bf16, tag="expT")
                nc.scalar.copy(expT, expT_ps)
                wscr = dram_pool.tile([K, P], bf16, tag="wscr")
                nc.sync.dma_start(out=wscr, in_=expT)
                wrep = mlp_pool.tile([EMB, K * P], bf16, tag="wrep")
                nc.sync.dma_start(
                    out=wrep,
                    in_=wscr[None, :, :].broadcast_to([EMB, K, P]),
                )
                return dict(chunks=chunks, m_=m_, wrep=wrep, rden=rden)

            def stage_b(t, st):
                """extract coords, MLP, weighted sum, store."""
                mx = mid_pool.tile([P, K * CHF], f32, tag="mx")
                nc.vector.tensor_tensor(
                    out=mx, in0=st["m_"], in1=st["chunks"], op=Alu.mult
                )
                topA2 = sel_pool.tile([P, 2 * K], f32, tag="topA2")
                nc.vector.tensor_reduce(
                    out=topA2.rearrange("p (k c) -> p k c", c=2),
                    in_=mx.rearrange("p (k j32 c) -> p k c j32", k=K, c=2),
                    axis=mybir.AxisListType.X,
                    op=Alu.add,
                )

                topAT_ps = pspool.tile([2 * K, P], f32, tag="topat_ps")
                nc.tensor.transpose(topAT_ps, topA2, ident)
                topAT = sel_pool.tile([2 * K, P], f32, tag="topAT")
                nc.scalar.copy(topAT, topAT_ps)

                psum1 = ppool.tile([EMB, K * P], f32, tag="pbig1")
                for k in range(K):
                    nc.tensor.matmul(
                        psum1[:, ts(k, P)], w1big[:, ts(k, EMB)], topAT
                    )
                h1 = mlp_pool.tile([EMB, K * P], bf16, tag="h1")
                nc.scalar.activation(h1, psum1, Act.Gelu, bias=b1c)

                psum2 = ppool.tile([EMB, K * P], f32, tag="pbig2")
                nc.tensor.matmul(psum2[:, :512], w2T, h1[:, :512])
                nc.tensor.matmul(psum2[:, 512:], w2T, h1[:, 512:])
                topE = mlp_pool.tile([EMB, K * P], bf16, tag="topE")
                nc.scalar.activation(topE, psum2, Act.Gelu, bias=b2c)

                wtmp = mlp_pool.tile([EMB, K * P], bf16, tag="wtmp")
                nc.vector.tensor_tensor(
                    out=wtmp, in0=topE, in1=st["wrep"], op=Alu.mult
                )
                f1 = mlp_pool.tile([EMB, 4 * P], bf16, tag="f1")
                nc.vector.tensor_tensor(
                    out=f1, in0=wtmp[:, : 4 * P], in1=wtmp[:, 4 * P :], op=Alu.add
                )
                f2 = sel_pool.tile([EMB, 2 * P], bf16, tag="f2")
                nc.vector.tensor_tensor(
                    out=f2, in0=f1[:, : 2 * P], in1=f1[:, 2 * P :], op=Alu.add
                )
                outT = sel_pool.tile([EMB, P], f32, tag="outT")
                nc.vector.tensor_tensor(
                    out=outT, in0=f2[:, :P], in1=f2[:, P:], op=Alu.add
                )

                outQ_ps = pspool.tile([P, EMB], f32, tag="outq_ps")
                nc.tensor.transpose(outQ_ps, outT, ident)
                out_sb = sel_pool.tile([P, EMB], f32, tag="out_sb")
                nc.scalar.mul(out_sb, outQ_ps, mul=st["rden"])

                nc.sync.dma_start(out=out_ext[ts(t, P), :], in_=out_sb)

            DEPTH = 3
            state = {}
            for t in range(n_tiles + DEPTH):
                if t < n_tiles:
                    state[t] = stage_a(t)
                if t >= DEPTH:
                    stage_b(t - DEPTH, state.pop(t - DEPTH))

    nc.compile()
    return nc


def make_in_map(gl_shard, anc_shard, prep, n_tiles):
    m = {
        "ngl": shard_queries(gl_shard, n_tiles),
        "anc": anc_shard,
    }
    m.update(prep)
    return m


_GRAPH_CACHE = {}
_TRACE = False       # set by test harnesses to capture a profile
LAST_RESULT = None   # BassKernelResults of the most recent kernel() call


def kernel(Gl_cur, ancL, W1, b1, W2, b2):
    global LAST_RESULT
    from concourse.bass_utils import run_bass_kernel_spmd

    Gl_cur = np.ascontiguousarray(Gl_cur, dtype=np.float32)
    ancL = np.ascontiguousarray(ancL, dtype=np.float32)
    prep = host_prep(
        np.asarray(W1, dtype=np.float32),
        np.asarray(b1, dtype=np.float32),
        np.asarray(W2, dtype=np.float32),
        np.asarray(b2, dtype=np.float32),
    )

    if "nc" not in _GRAPH_CACHE:
        _GRAPH_CACHE["nc"] = build_graph(NT_FULL)
    nc = _GRAPH_CACHE["nc"]

    in_maps = []
    for i in range(NCORES):
        sl = slice(i * BP, (i + 1) * BP)
        in_maps.append(make_in_map(Gl_cur[sl], ancL[sl], prep, NT_FULL))
    res = run_bass_kernel_spmd(nc, in_maps, list(range(NCORES)), trace=_TRACE)
    LAST_RESULT = res
    return np.concatenate([res.results[i]["out"] for i in range(NCORES)], axis=0)
